# revision 1
# baseline (speedup 1.0000x reference)
"""Trainium2 Bass kernel for KeypointSelector:
conv3x3(384->128, pad 1) + bias + ReLU -> conv1x1(128->1) + bias + sigmoid.

Input  dino_features: (32, 64, 64, 384) f32
Output (32, 64, 64, 1) f32

Strategy: pure data parallel over batch, 4 images per core on 8 cores.
Conv3x3 is computed as 27 accumulating matmuls (9 taps x 3 cin chunks of 128)
on the PE array in bf16 (fp32 PSUM accumulation), software-pipelined by one
tile so the ACT-engine ReLU never stalls the PE. Input is laid out host-side
as [cin, padded_pixel] so matmul taps are just column-shifted slices of a
contiguous padded pixel axis.
"""

import ml_dtypes
import numpy as np

import concourse.bass as bass
import concourse.tile as tile
from concourse import bacc, mybir
from concourse.bass_utils import run_bass_kernel_spmd

BF16 = ml_dtypes.bfloat16

# Geometry
B, H, W, CIN, CHID = 32, 64, 64, 384, 128
NCORES = 8
BLOC = B // NCORES  # 4 images per core
HP, WP = H + 2, W + 2  # 66x66 padded grid
NPIX = HP * WP  # 4356 padded pixels per image
TS = 512  # matmul free-dim tile (one PSUM bank of fp32)
START = WP + 1  # padded idx of first valid output pixel (1,1) = 67
END = H * WP + W + 1  # 4289: one past padded idx (h+1)*WP+(w+1) of pixel (63,63)
NT = -(-(END - START) // TS)  # 9 tiles per image (last one partial)
TILE_N = [min(TS, END - START - t * TS) for t in range(NT)]  # [512]*8 + [126]
PIXBUF = NPIX  # taps of valid pixels stay within the padded image
OUTW = 64 * WP  # out_s columns actually read by the output DMA (4224)
NCHUNK = CIN // 128  # 3 cin chunks

_CACHED = {}


def _build_bass(reps=1, reload=True):
    nc = bacc.Bacc("TRN2", target_bir_lowering=False)

    f32 = mybir.dt.float32
    bf16 = mybir.dt.bfloat16

    x = nc.dram_tensor("x", [BLOC, NCHUNK, 128, PIXBUF], bf16,
                       kind="ExternalInput")
    w1 = nc.dram_tensor("w1", [NCHUNK, 128, 9, CHID], bf16,
                        kind="ExternalInput")
    b1 = nc.dram_tensor("b1", [CHID, 1], mybir.dt.float32, kind="ExternalInput")
    w2 = nc.dram_tensor("w2", [CHID, 1], bf16, kind="ExternalInput")
    b2 = nc.dram_tensor("b2", [1, 1], mybir.dt.float32, kind="ExternalInput")
    y = nc.dram_tensor("y", [BLOC, H, W], mybir.dt.float32, kind="ExternalOutput")

    with tile.TileContext(nc) as tc:
        with (
            tc.tile_pool(name="consts", bufs=1) as consts,
            tc.tile_pool(name="xin", bufs=3 if reload else 1) as xin,
            tc.tile_pool(name="hbuf", bufs=3) as hbuf,
            tc.tile_pool(name="obuf", bufs=2) as obuf,
            tc.tile_pool(name="ps1", bufs=2, space="PSUM") as ps1,
            tc.tile_pool(name="ps2", bufs=2, space="PSUM") as ps2,
        ):
            # Constants: conv weights + biases, resident for the whole kernel
            w1_s = consts.tile([128, NCHUNK, 9, CHID], bf16)
            for ch in range(NCHUNK):
                nc.sync.dma_start(out=w1_s[:, ch, :, :], in_=w1[ch])
            b1_s = consts.tile([CHID, 1], f32)
            nc.sync.dma_start(out=b1_s, in_=b1[:])
            w2_s = consts.tile([CHID, 1], bf16)
            nc.sync.dma_start(out=w2_s, in_=w2[:])
            b2_s = consts.tile([1, 1], f32)
            nc.sync.dma_start(out=b2_s, in_=b2[:])

            # One-tile software pipeline: the 1x1 matmul for tile t-1 is
            # emitted after tile t's conv matmuls, so PE never waits on the
            # ACT ReLU. `pend` carries (h_s, out_s, col, n, last_of_image).
            pend = None

            def flush(pend):
                h_p, out_p, col, n, img = pend
                p2 = ps2.tile([1, TS], f32)
                nc.tensor.matmul(out=p2[0:1, :n], lhsT=w2_s[:], rhs=h_p[:, :n],
                                 start=True, stop=True)
                nc.scalar.activation(
                    out=out_p[0:1, col:col + n], in_=p2[0:1, :n],
                    func=mybir.ActivationFunctionType.Sigmoid,
                    bias=b2_s[0:1], scale=1.0,
                )
                if img is not None:
                    # Image done: write back the valid 64x64 pixels. Padded
                    # idx of (h,w) is START + 66*h + w -> out_s col 66*h + w.
                    src = out_p[0:1, :OUTW].rearrange("p (h w) -> p h w", w=WP)
                    nc.sync.dma_start(out=y[img], in_=src[:, :, 0:W])

            preloaded = {}
            if not reload:  # benchmark mode: load all images once up front
                for i in range(BLOC):
                    for ch in range(NCHUNK):
                        xc = xin.tile([128, PIXBUF], bf16, tag=f"xp{i}_{ch}")
                        nc.sync.dma_start(out=xc[:], in_=x[i, ch])
                        preloaded[(i, ch)] = xc

            for i in [ii for _ in range(reps) for ii in range(BLOC)]:
                # One tile per cin chunk: conv matmuls on chunk 0 start as
                # soon as its DMA lands, not after all three.
                xs = []
                # Spread the chunk streams over both HWDGE queues (SP, ACT)
                # so they transfer in parallel.
                dma_eng = [nc.sync, nc.scalar, nc.sync]
                for ch in range(NCHUNK):
                    if not reload:
                        xs.append(preloaded[(i, ch)])
                        continue
                    xc = xin.tile([128, PIXBUF], bf16, tag=f"xs{ch}")
                    # Column segments per chunk: the first tile's columns
                    # arrive sooner than with a monolithic chunk DMA. Fewer
                    # segments on the ACT queue keep its enqueue cost low.
                    nseg = 2 if dma_eng[ch] is nc.scalar else 4
                    seg = -(-PIXBUF // nseg)
                    for g in range(nseg):
                        a, b = g * seg, min((g + 1) * seg, PIXBUF)
                        dma_eng[ch].dma_start(out=xc[:, a:b], in_=x[i, ch, :, a:b])
                    xs.append(xc)

                out_s = obuf.tile([1, OUTW], f32)
                for t in range(NT):
                    n = TILE_N[t]
                    s0 = START + t * TS
                    p1 = ps1.tile([CHID, TS], f32)
                    k = 0
                    for ch in range(NCHUNK):
                        for dy in (-1, 0, 1):
                            for dx in (-1, 0, 1):
                                tap = (dy + 1) * 3 + (dx + 1)
                                s = s0 + dy * WP + dx
                                nc.tensor.matmul(
                                    out=p1[:, :n],
                                    lhsT=w1_s[:, ch, tap, :],
                                    rhs=xs[ch][:, s:s + n],
                                    start=(k == 0),
                                    stop=(k == 9 * NCHUNK - 1),
                                )
                                k += 1
                    if pend is not None:
                        flush(pend)
                    # h = relu(conv + b1), rounded to bf16 for the 1x1 matmul
                    h_s = hbuf.tile([CHID, TS], bf16)
                    nc.scalar.activation(
                        out=h_s[:, :n], in_=p1[:, :n],
                        func=mybir.ActivationFunctionType.Relu,
                        bias=b1_s[:], scale=1.0,
                    )
                    pend = (h_s, out_s, t * TS, n,
                            i if t == NT - 1 else None)
            flush(pend)
    nc.compile()
    return nc


def _prep_inputs(dino_features, W1, b1, W2, b2):
    xp = np.zeros((B, HP, WP, CIN), dtype=np.float32)
    xp[:, 1:H + 1, 1:W + 1, :] = dino_features
    # -> [B, cin, padded_pixel], chunked cin
    xt = xp.transpose(0, 3, 1, 2).reshape(B, NCHUNK, 128, NPIX)
    xbuf = np.zeros((B, NCHUNK, 128, PIXBUF), dtype=BF16)
    xbuf[:, :, :, :NPIX] = xt.astype(BF16)

    # W1 (3,3,384,128) (ky,kx,ci,co) -> [chunk, cin_in_chunk, tap, cout]
    w1h = np.ascontiguousarray(
        W1.transpose(2, 0, 1, 3).reshape(NCHUNK, 128, 9, CHID).astype(BF16))
    b1h = np.ascontiguousarray(b1.reshape(CHID, 1).astype(np.float32))
    w2h = np.ascontiguousarray(W2.reshape(CHID, 1).astype(BF16))
    b2h = np.ascontiguousarray(b2.reshape(1, 1).astype(np.float32))

    in_maps = []
    for c in range(NCORES):
        in_maps.append({
            "x": np.ascontiguousarray(xbuf[c * BLOC:(c + 1) * BLOC]),
            "w1": w1h, "b1": b1h, "w2": w2h, "b2": b2h,
        })
    return in_maps


def kernel(dino_features, W1, b1, W2, b2, _trace=False, _trace_kwargs=None):
    if "nc" not in _CACHED:
        _CACHED["nc"] = _build_bass()
    nc = _CACHED["nc"]
    in_maps = _prep_inputs(dino_features, W1, b1, W2, b2)
    res = run_bass_kernel_spmd(nc, in_maps, core_ids=list(range(NCORES)),
                               trace=_trace, **(_trace_kwargs or {}))
    _CACHED["last_results"] = res
    out = np.concatenate([res.results[c]["y"] for c in range(NCORES)], axis=0)
    return out.reshape(B, H, W, 1).astype(np.float32)



# revision 5
# speedup vs baseline: 2.8301x; 2.8301x over previous
"""Trainium2 Bass kernel for KeypointSelector:
conv3x3(384->128, pad 1) + bias + ReLU -> conv1x1(128->1) + bias + sigmoid.

Input  dino_features: (32, 64, 64, 384) f32
Output (32, 64, 64, 1) f32

Strategy: pure data parallel over batch, 4 images per core on 8 cores.
Conv3x3 runs on the PE array in fp8e4m3 with MatmulPerfMode.DoubleRow:
each matmul contracts TWO 128-deep k-tiles (cin-chunk x tap pairs) at
0.5 cycles per output column -- 4x the bf16 column rate. The 27 k-tiles
(3 cin chunks x 9 taps) are padded to 28 and paired so that both slices
of every pair live at a constant column offset (delta) inside one flat
[128, 3*4356] SBUF image tile; all deltas exceed the 512-column moving
tile so no access pattern overlaps.

fp8 scaling: x is pre-scaled by 16 and w1 by 256 host-side (w1 values
~1e-2 would land in the fp8 subnormal range unscaled); the 1/4096
descale is folded into the ReLU activation's scale argument. The 1x1
conv + sigmoid stay in bf16/f32. Measured end-to-end rel err ~6e-3.

Weight-reuse groups: tiles are processed in groups of 3 with the pair
loop outermost (k-outer, tile-inner), so each DoubleRow LDWEIGHTS serves
3 matmuls and stays hidden under the previous matmul on hardware.
"""

import ml_dtypes
import numpy as np

import concourse.bass as bass
import concourse.tile as tile
from concourse import bacc, mybir
from concourse.ap import AP
from concourse.bass_utils import run_bass_kernel_spmd

BF16 = ml_dtypes.bfloat16
E4M3 = ml_dtypes.float8_e4m3  # == mybir.dt.np(float8e4)

# Geometry
B, H, W, CIN, CHID = 32, 64, 64, 384, 128
NCORES = 8
BLOC = B // NCORES  # 4 images per core
HP, WP = H + 2, W + 2  # 66x66 padded grid
NPIX = HP * WP  # 4356 padded pixels per image
TS = 512  # matmul free-dim tile (one PSUM bank of fp32)
START = WP + 1  # padded idx of first valid output pixel (1,1) = 67
END = H * WP + W + 1  # 4289: one past padded idx of pixel (63,63)
NT = -(-(END - START) // TS)  # 9 tiles per image (last one partial)
TILE_N = [min(TS, END - START - t * TS) for t in range(NT)]  # [512]*8 + [126]
NCHUNK = CIN // 128  # 3 cin chunks
GROUPS = [[0, 1, 2], [3, 4, 5], [6, 7, 8]]  # weight-reuse tile groups
OUTW = NT * TS  # padded flat output row (4608); only first 4222 cols valid
OUTV = END - START  # 4222 valid flat output cols

# fp8 scaling
SX, SW = 16.0, 256.0
DESCALE = 1.0 / (SX * SW)

# k-tile pairing for DoubleRow: 27 (chunk, tap) k-tiles + 1 zero slot -> 14
# pairs. Tap t=(dy+1)*3+(dx+1) reads at padded-col offset OFF[t]; chunk c
# lives at flat offset c*NPIX. Pairs are chosen cross-chunk so every rhs
# delta is >= NPIX-134 > 512 (no overlapping access patterns).
OFF = [-WP - 1, -WP, -WP + 1, -1, 0, 1, WP - 1, WP, WP + 1]
PAIRS = (
    [(0, t, 1, t) for t in range(5)]
    + [(0, 5 + j, 2, j) for j in range(4)]
    + [(1, 5 + j, 2, 4 + j) for j in range(4)]
    + [(None, None, 2, 8)]  # slot A holds zero weights
)
NPAIR = len(PAIRS)  # 14
BASEA, DELTA = [], []
for cA, tA, cB, tB in PAIRS:
    if cA is None:
        # zero-weight A slice: point at valid chunk-1 data, delta to chunk 2
        BASEA.append(1 * NPIX + OFF[tB])
        DELTA.append(NPIX)
    else:
        BASEA.append(cA * NPIX + OFF[tA])
        DELTA.append((cB - cA) * NPIX + OFF[tB] - OFF[tA])
assert all(d > TS for d in DELTA)

_CACHED = {}


def _build_bass(reps=1):
    nc = bacc.Bacc("TRN2", target_bir_lowering=False)

    f32 = mybir.dt.float32
    bf16 = mybir.dt.bfloat16
    fp8 = mybir.dt.float8e4
    DR = mybir.MatmulPerfMode.DoubleRow

    x = nc.dram_tensor("x", [BLOC, NCHUNK, 128, NPIX], fp8, kind="ExternalInput")
    w1 = nc.dram_tensor("w1", [128, NPAIR, 2, CHID], fp8, kind="ExternalInput")
    b1 = nc.dram_tensor("b1", [CHID, 1], f32, kind="ExternalInput")
    w2 = nc.dram_tensor("w2", [CHID, 1], bf16, kind="ExternalInput")
    b2 = nc.dram_tensor("b2", [1, 1], f32, kind="ExternalInput")
    y = nc.dram_tensor("y", [BLOC, OUTW], f32, kind="ExternalOutput")

    with tile.TileContext(nc) as tc:
        with (
            tc.tile_pool(name="consts", bufs=1) as consts,
            tc.tile_pool(name="xin", bufs=2) as xin,
            tc.tile_pool(name="hbuf", bufs=4) as hbuf,
            tc.tile_pool(name="obuf", bufs=2) as obuf,
            tc.tile_pool(name="ps1", bufs=2, space="PSUM") as ps1,
            tc.tile_pool(name="ps2", bufs=2, space="PSUM") as ps2,
        ):
            # Constants: paired conv weights + biases, resident throughout
            w1_s = consts.tile([128, NPAIR, 2, CHID], fp8)
            nc.sync.dma_start(out=w1_s[:], in_=w1[:])
            b1_s = consts.tile([CHID, 1], f32)
            nc.sync.dma_start(out=b1_s, in_=b1[:])
            w2_s = consts.tile([CHID, 1], bf16)
            nc.sync.dma_start(out=w2_s, in_=w2[:])
            b2_s = consts.tile([1, 1], f32)
            nc.sync.dma_start(out=b2_s, in_=b2[:])

            # One-group software pipeline: group g's ReLUs/1x1s are emitted
            # after group g+1's conv matmuls so the PE never queues behind
            # the ACT engine. `pend` carries [(h_s, n, col)], out_s, img.
            pend = None

            def flush(pend):
                tiles, out_s, img = pend
                for h_s, n, col in tiles:
                    p2 = ps2.tile([1, TS], f32)
                    nc.tensor.matmul(out=p2[0:1, :n], lhsT=w2_s[:],
                                     rhs=h_s[:, :n], start=True, stop=True)
                    nc.scalar.activation(
                        out=out_s[0:1, col:col + n], in_=p2[0:1, :n],
                        func=mybir.ActivationFunctionType.Sigmoid,
                        bias=b2_s[0:1], scale=1.0,
                    )
                if img is not None:
                    nc.sync.dma_start(out=y[img, 0:OUTV],
                                      in_=out_s[0:1, 0:OUTV])

            for i in [ii for _ in range(reps) for ii in range(BLOC)]:
                # Flat fp8 image tile: 3 cin chunks side by side.
                xf = xin.tile([128, NCHUNK * NPIX], fp8, tag="x")
                pstride = list(xf.ap)[0][0]
                # Two column segments per chunk across two DGE queues so the
                # first group's columns land early.
                half = NPIX // 2
                for seg in range(2):
                    a, b = seg * half, (seg + 1) * half if seg == 0 else NPIX
                    for ch in range(NCHUNK):
                        eng = nc.gpsimd if ch == 1 else nc.sync
                        eng.dma_start(out=xf[:, ch * NPIX + a:ch * NPIX + b],
                                      in_=x[i, ch, :, a:b])

                out_s = obuf.tile([1, OUTW], f32)
                for grp in GROUPS:
                    ptiles = [ps1.tile([CHID, TS], f32, tag=f"p{g % 3}",
                                       name=f"p1_{g % 3}")
                              for g in grp]
                    # k-outer / tile-inner: one weight load per pair serves
                    # the whole group.
                    for k in range(NPAIR):
                        for g, t in enumerate(grp):
                            n = TILE_N[t]
                            s0 = START + t * TS
                            rhs = AP(xf.tensor, xf.offset + BASEA[k] + s0,
                                     [[pstride, 128], [DELTA[k], 2], [1, n]])
                            nc.tensor.matmul(
                                out=ptiles[g][:, :n],
                                lhsT=w1_s[:, k],
                                rhs=rhs,
                                start=(k == 0),
                                stop=(k == NPAIR - 1),
                                perf_mode=DR,
                            )
                    if pend is not None:
                        flush(pend)
                    # h = relu(conv/4096 + b1), rounded to bf16 for the 1x1
                    tiles = []
                    for g, t in enumerate(grp):
                        n = TILE_N[t]
                        h_s = hbuf.tile([CHID, TS], bf16, tag=f"h{g % 3}")
                        nc.scalar.activation(
                            out=h_s[:, :n], in_=ptiles[g][:, :n],
                            func=mybir.ActivationFunctionType.Relu,
                            bias=b1_s[:], scale=DESCALE,
                        )
                        tiles.append((h_s, n, t * TS))
                    pend = (tiles, out_s,
                            i if grp is GROUPS[-1] else None)
            flush(pend)
    nc.compile()
    return nc


def _prep_inputs(dino_features, W1, b1, W2, b2):
    xp = np.zeros((B, HP, WP, CIN), dtype=np.float32)
    xp[:, 1:H + 1, 1:W + 1, :] = dino_features * SX
    # -> [B, chunk, cin_in_chunk, padded_pixel]
    xq = np.ascontiguousarray(
        xp.transpose(0, 3, 1, 2).reshape(B, NCHUNK, 128, NPIX)).astype(E4M3)

    # W1 (3,3,384,128) (ky,kx,ci,co) -> [chunk, cin128, tap, cout], then pair
    wq = (np.asarray(W1) * SW).astype(E4M3)
    wr = wq.transpose(2, 0, 1, 3).reshape(NCHUNK, 128, 9, CHID)
    w1p = np.zeros((128, NPAIR, 2, CHID), dtype=E4M3)
    for k, (cA, tA, cB, tB) in enumerate(PAIRS):
        if cA is not None:
            w1p[:, k, 0, :] = wr[cA, :, tA, :]
        w1p[:, k, 1, :] = wr[cB, :, tB, :]

    b1h = np.ascontiguousarray(b1.reshape(CHID, 1).astype(np.float32))
    w2h = np.ascontiguousarray(W2.reshape(CHID, 1).astype(BF16))
    b2h = np.ascontiguousarray(b2.reshape(1, 1).astype(np.float32))

    in_maps = []
    for c in range(NCORES):
        in_maps.append({
            "x": np.ascontiguousarray(xq[c * BLOC:(c + 1) * BLOC]),
            "w1": w1p, "b1": b1h, "w2": w2h, "b2": b2h,
        })
    return in_maps


def kernel(dino_features, W1, b1, W2, b2, _trace=False, _trace_kwargs=None):
    if "nc" not in _CACHED:
        _CACHED["nc"] = _build_bass()
    nc = _CACHED["nc"]
    in_maps = _prep_inputs(dino_features, W1, b1, W2, b2)
    res = run_bass_kernel_spmd(nc, in_maps, core_ids=list(range(NCORES)),
                               trace=_trace, **(_trace_kwargs or {}))
    _CACHED["last_results"] = res
    out = np.concatenate([res.results[c]["y"] for c in range(NCORES)], axis=0)
    # Flat col 66*h + w (w<64) -> pixel (h, w); cols >= OUTV are padding.
    out = out[:, :H * WP].reshape(B, H, WP)[:, :, :W]
    return np.ascontiguousarray(out).reshape(B, H, W, 1).astype(np.float32)
